# revision 23
# baseline (speedup 1.0000x reference)
"""Trainium2 Bass kernel for pointer-generator additive attention.

Full op (per batch b):
    dec_fea = s_t_hat @ W_d.T + b_d                         # (n,)
    att     = EF[b] + dec_fea[None,:] + cov[b][:,None]*W_c  # (t, n)
    score   = tanh(att) @ v                                 # (t,)
    attn    = renorm(softmax(score) * mask)                 # (t,)
    c_t     = attn @ EO[b]                                  # (n,)
    cov_next= cov + attn

Data-parallel over batch across 8 NeuronCores (8 batches/core, params
replicated, no collectives).

HBM strategy: EF (n-major transposed) and EO (t-major) are cast to bf16 and
permuted on the host into partition-contiguous layouts; each batch is ONE
2 MB HWDGE dma_start with 16 KB per-partition lines. The big stream owns the
nc.sync ring in FIFO need-order: W_d(fp8), EF0, EO0, EF1, ...
Total ~33 MB/core -> ~92 us DMA floor.

Engine split (att is n-major: partition = n, free = t):
  - DVE stt:  att_pre = EFT + W_c[n]*cov_bcast  (W_c as per-partition scalar)
  - ScalarE:  th = tanh(att_pre + dec[n])       (dec as per-partition bias)
  - PE score: M=1 matvecs (lhsT = v column per n-tile) accumulating a
    [1,1024] PSUM row over the 8 n-tiles.
  - PE c_t:   M=1 matvecs (lhsT = attn column, rhs = t-major EO tiles),
    interleaved tile-by-tile with the next batch's score matmuls.
  - softmax on an [8,128] grid: one exp (ScalarE), masked-sum via stt accum
    (DVE), total via a tiny PE ones-matmul -> [128,1] reciprocal, attn
    columns for c_t via one PE transpose of the unnormalized e-grid
    (short dependency chain so the c_t matmuls never head-block the PE).
dec_fea is a tiny fp8 PE matmul at init; cov broadcasts / dec columns are
prebuilt during the initial DMA ramp. GpSimd does only small DMAs (its
elementwise ops measured 10-25x slower than DVE and stall DVE via the
shared SBUF port; PE matvecs sit at the HAM-throttled 1.2 GHz clock since
M=1 registers almost no array activity - both measured dead ends).
"""

import sys

if "/opt/trn_rl_repo" not in sys.path:
    sys.path.insert(0, "/opt/trn_rl_repo")

import ml_dtypes
import numpy as np

import concourse.bass as bass
import concourse.mybir as mybir
import concourse.tile as tile
from concourse import bacc
from concourse.bass_utils import run_bass_kernel_spmd
from concourse.masks import make_identity

F32 = mybir.dt.float32
BF16 = mybir.dt.bfloat16
FP8 = mybir.dt.float8e4
AF = mybir.ActivationFunctionType
ALU = mybir.AluOpType

N_CORES = 8
B = 64
NB = B // N_CORES  # local batches per core
T = 1024
N = 1024
TJ = T // 128       # 128-tiles per batch (both t- and n- direction)
KT = N // 128       # k-tiles for the W_d matvec


def build_bass(nb: int = NB) -> bass.Bass:
    nc = bacc.Bacc()

    eft_d = nc.declare_dram_parameter("eft_bf16", [nb, 128, TJ, T], BF16, isOutput=False)
    eo_d = nc.declare_dram_parameter("eo_bf16", [nb, 128, TJ, N], BF16, isOutput=False)
    mk_d = nc.declare_dram_parameter("enc_padding_mask", [nb, T], F32, isOutput=False)
    cv_d = nc.declare_dram_parameter("coverage", [nb, T], F32, isOutput=False)
    wdt_d = nc.declare_dram_parameter("W_d_T", [N, N], FP8, isOutput=False)
    st_d = nc.declare_dram_parameter("s_t_hat_T", [N, nb], FP8, isOutput=False)
    bd_d = nc.declare_dram_parameter("b_d", [N], BF16, isOutput=False)
    wc_d = nc.declare_dram_parameter("W_c", [N], BF16, isOutput=False)
    v_d = nc.declare_dram_parameter("v", [N], BF16, isOutput=False)
    vdg_d = nc.declare_dram_parameter("vdiag", [128, TJ, 128], BF16, isOutput=False)
    ct_o = nc.declare_dram_parameter("c_t", [nb, N], F32, isOutput=True)
    at_o = nc.declare_dram_parameter("attn", [nb, T], F32, isOutput=True)
    cn_o = nc.declare_dram_parameter("coverage_next", [nb, T], F32, isOutput=True)

    with tile.TileContext(nc) as tc:
        with (
            tc.tile_pool(name="consts", bufs=1) as consts,
            tc.tile_pool(name="wdtp", bufs=1) as wdtp,
            tc.tile_pool(name="covp", bufs=1) as covp,
            tc.tile_pool(name="efp", bufs=3) as efp,
            tc.tile_pool(name="eop", bufs=3) as eop,
            tc.tile_pool(name="attp", bufs=3) as attp,
            tc.tile_pool(name="thp", bufs=3) as thp,
            tc.tile_pool(name="smal", bufs=4) as smal,
            tc.tile_pool(name="rowstg", bufs=2) as rowstg,
            tc.tile_pool(name="psP", bufs=2, space="PSUM") as psP,
            tc.tile_pool(name="psC", bufs=1, space="PSUM") as psC,
            tc.tile_pool(name="psT", bufs=1, space="PSUM") as psT,
        ):
            # ---------------- big-stream DMAs (sync ring, FIFO need-order) ----
            wdt_all = wdtp.tile([128, KT, N], FP8)
            nc.sync.dma_start(
                out=wdt_all, in_=wdt_d.rearrange("(kj p) n -> p kj n", p=128)
            )
            ef_bufs = {}
            eo_bufs = {}

            def load_ef(b, split=1):
                t = efp.tile([128, TJ, T], BF16, tag="ef", name=f"ef{b}")
                step = TJ // split
                for s in range(split):
                    nc.sync.dma_start(
                        out=t[:, s * step:(s + 1) * step, :],
                        in_=eft_d[b, :, s * step:(s + 1) * step, :],
                    )
                ef_bufs[b] = t

            def load_eo(b):
                t = eop.tile([128, TJ, N], BF16, tag="eo", name=f"eo{b}")
                nc.sync.dma_start(out=t, in_=eo_d[b])
                eo_bufs[b] = t

            load_ef(0, split=2)
            load_eo(0)
            load_ef(1)
            load_eo(1)

            # ---------------- constants / small inputs (gpsimd ring) ----------
            ident = consts.tile([128, 128], F32)
            make_identity(nc, ident)
            ident_b = consts.tile([128, 128], BF16)
            nc.vector.tensor_copy(ident_b, ident)
            ones_f32 = consts.tile([1, T], F32)
            nc.vector.memset(ones_f32, 1.0)
            ones_b = consts.tile([1, T], BF16)
            nc.vector.memset(ones_b, 1.0)
            ones8w = consts.tile([TJ, 128], F32)
            nc.vector.memset(ones8w, 1.0)
            ones8b = consts.tile([TJ, 1], BF16)
            nc.vector.memset(ones8b, 1.0)

            sT_all = consts.tile([128, KT, 32], FP8)
            nc.gpsimd.dma_start(
                out=sT_all[:, :, 0:nb],
                in_=st_d.rearrange("(kj p) b -> p kj b", p=128),
            )
            bd_b = consts.tile([1, N], BF16)
            nc.gpsimd.dma_start(out=bd_b, in_=bd_d[None, :])
            v8 = consts.tile([TJ, 128], BF16)
            nc.gpsimd.dma_start(out=v8, in_=v_d.rearrange("(j t) -> j t", j=TJ))
            wc8 = consts.tile([TJ, 128], BF16)
            nc.gpsimd.dma_start(out=wc8, in_=wc_d.rearrange("(j t) -> j t", j=TJ))
            mk8_all = consts.tile([TJ, nb, 128], F32)
            nc.gpsimd.dma_start(
                out=mk8_all, in_=mk_d.rearrange("b (j t) -> j b t", j=TJ)
            )
            cov8_all = consts.tile([TJ, nb, 128], F32)
            nc.gpsimd.dma_start(
                out=cov8_all, in_=cv_d.rearrange("b (j t) -> j b t", j=TJ)
            )

            # v / W_c as per-partition columns: transpose [8,128] -> [128,8]
            v_cols = consts.tile([128, TJ], BF16)
            wc_cols = consts.tile([128, TJ], BF16)
            for src, dst in ((v8, v_cols), (wc8, wc_cols)):
                pst = psT.tile([128, TJ], BF16, tag="tscratchb")
                nc.tensor.matmul(
                    pst, lhsT=src, rhs=ident_b[0:TJ, 0:TJ], is_transpose=True,
                    start=True, stop=True,
                )
                nc.vector.tensor_copy(dst, pst)

            # vdiag[j] = [128,128] with only column j nonzero (the v-slice):
            # an M=128 stationary operand so score matvecs register full
            # array activity (keeps the HAM clock at 8/8)


            # dec_fea = s_t_hat @ W_d.T + b_d  (fp8 matmuls, tiny), then
            # transpose into per-partition columns dec_cols[:, j, b]
            dec_cols = consts.tile([128, TJ, nb], F32)
            for h in range(2):
                sl = slice(h * 512, (h + 1) * 512)
                psd = psT.tile([nb, 512], F32, tag="tscratch")
                for kj in range(KT):
                    nc.tensor.matmul(
                        psd, lhsT=sT_all[:, kj, 0:nb], rhs=wdt_all[:, kj, sl],
                        start=(kj == 0), stop=False,
                    )
                nc.tensor.matmul(
                    psd, lhsT=ones_b[0:1, 0:nb], rhs=bd_b[0:1, sl],
                    start=False, stop=True,
                )
                dec8 = smal.tile([nb, 512], F32, tag="dec8")
                nc.vector.tensor_copy(dec8, psd)
                for jj in range(4):
                    j = h * 4 + jj
                    pst = psT.tile([128, TJ], F32, tag="tscratch")
                    nc.tensor.matmul(
                        pst[:, 0:nb], lhsT=dec8[:, jj * 128:(jj + 1) * 128],
                        rhs=ident[0:nb, 0:nb], is_transpose=True,
                        start=True, stop=True,
                    )
                    nc.vector.tensor_copy(dec_cols[:, j, :], pst[:, 0:nb])

            # cov broadcast tiles for every batch (built during the DMA ramp)
            covb_all = covp.tile([128, nb, T], BF16)
            for b in range(nb):
                covrow = smal.tile([1, T], BF16, tag="covrow", name=f"cvr{b}")
                nc.gpsimd.dma_start(out=covrow, in_=cv_d[None, b, :])  # cast
                for h in range(2):
                    cps = psT.tile([128, 512], F32, tag="tscratch")
                    nc.tensor.matmul(
                        cps, lhsT=ones_b[0:1, 0:128],
                        rhs=covrow[0:1, h * 512:(h + 1) * 512],
                        start=True, stop=True,
                    )
                    nc.vector.tensor_copy(
                        covb_all[:, b, h * 512:(h + 1) * 512], cps
                    )

            # ---------------- main loop over local batches ----------------
            attn_cols = {}
            ct_ps = {}

            def ct_finish(b):
                ps = ct_ps.pop(b)
                ct_row = rowstg.tile([1, N], BF16, tag="ctb", name=f"ct{b}")
                nc.vector.tensor_copy(ct_row[:, 0:512], ps[:, 0:512])
                nc.vector.tensor_copy(ct_row[:, 512:1024], ps[:, 512:1024])
                nc.gpsimd.dma_start(out=ct_o[None, b, :], in_=ct_row)
                eo_bufs.pop(b)

            def softmax_block(b, spart):
                score_row = rowstg.tile([1, T], BF16, tag="score", name=f"sc{b}")
                nc.vector.tensor_copy(score_row[:, 0:512], spart[:, 0:512])
                nc.vector.tensor_copy(score_row[:, 512:1024], spart[:, 512:1024])
                score8 = smal.tile([TJ, 128], BF16, tag="s8")
                nc.gpsimd.dma_start(
                    out=score8,
                    in_=score_row[0:1, :].rearrange("p (j t) -> p j t", j=TJ),
                )
                e8 = smal.tile([TJ, 128], F32, tag="e8")
                # scores are O(1) (|s| < ~3): plain exp is safe, skip max-sub
                nc.scalar.activation(e8, score8, AF.Exp)
                e8m = smal.tile([TJ, 128], F32, tag="e8m")
                msum8 = smal.tile([TJ, 1], F32, tag="msum")
                nc.vector.scalar_tensor_tensor(
                    out=e8m, in0=e8, scalar=1.0, in1=mk8_all[:, b, :],
                    op0=ALU.mult, op1=ALU.mult, accum_out=msum8,
                )
                # S on all 128 partitions via a tiny ones-matmul, then 1/S
                s128 = psT.tile([128, 1], F32, tag="tscratchb")
                nc.tensor.matmul(
                    s128, lhsT=ones8w, rhs=msum8, start=True, stop=True,
                )
                rs128 = smal.tile([128, 1], F32, tag="rs128")
                nc.vector.reciprocal(rs128, s128)
                # attn columns for c_t: transpose unnormalized e-grid, scale
                ecp = psT.tile([128, TJ], F32, tag="tscratch")
                nc.tensor.matmul(
                    ecp, lhsT=e8m, rhs=ident[0:TJ, 0:TJ], is_transpose=True,
                    start=True, stop=True,
                )
                acols = smal.tile([128, TJ], BF16, tag="acols")
                nc.vector.tensor_scalar_mul(acols, ecp, rs128)
                attn_cols[b] = acols
                # attn / coverage outputs
                attn8 = smal.tile([TJ, 128], F32, tag="attn8")
                nc.vector.tensor_scalar_mul(attn8, e8m, rs128[0:TJ, :])
                covn8 = smal.tile([TJ, 128], F32, tag="covn8")
                nc.vector.tensor_add(covn8, cov8_all[:, b, :], attn8)
                nc.gpsimd.dma_start(
                    out=at_o[b].rearrange("(j t) -> j t", j=TJ), in_=attn8
                )
                nc.gpsimd.dma_start(
                    out=cn_o[b].rearrange("(j t) -> j t", j=TJ), in_=covn8
                )

            for b in range(nb):
                eft = ef_bufs.pop(b)
                if b + 2 < nb:
                    load_ef(b + 2)
                    load_eo(b + 2)
                spart = psP.tile([1, T], F32, tag="spart", name=f"spart{b}")
                if b > 0:
                    ct_ps[b - 1] = psC.tile([1, N], F32, tag="ctp",
                                            name=f"ctp{b}")
                for j in range(TJ):
                    att_pre = attp.tile([128, T], BF16, tag="attp")
                    nc.vector.scalar_tensor_tensor(
                        out=att_pre, in0=covb_all[:, b, :],
                        scalar=wc_cols[:, j:j + 1], in1=eft[:, j, :],
                        op0=ALU.mult, op1=ALU.add,
                    )
                    th = thp.tile([128, T], BF16, tag="th")
                    nc.scalar.activation(
                        th, att_pre, AF.Tanh, bias=dec_cols[:, j, b:b + 1]
                    )
                    for h in range(2):
                        nc.tensor.matmul(
                            spart[:, h * 512:(h + 1) * 512],
                            lhsT=v_cols[:, j:j + 1],
                            rhs=th[:, h * 512:(h + 1) * 512],
                            start=(j == 0), stop=(j == TJ - 1),
                            skip_group_check=True,
                        )
                    if b > 0:
                        acols = attn_cols[b - 1]
                        eot = eo_bufs[b - 1]
                        for h in range(2):
                            nc.tensor.matmul(
                                ct_ps[b - 1][:, h * 512:(h + 1) * 512],
                                lhsT=acols[:, j:j + 1],
                                rhs=eot[:, j, h * 512:(h + 1) * 512],
                                start=(j == 0), stop=(j == TJ - 1),
                                skip_group_check=True,
                            )
                softmax_block(b, spart)
                if b > 0:
                    ct_finish(b - 1)

            # trailing c_t for the last batch
            b = nb - 1
            ct_ps[b] = psC.tile([1, N], F32, tag="ctp", name="ctp_last")
            acols = attn_cols[b]
            eot = eo_bufs[b]
            for j in range(TJ):
                for h in range(2):
                    nc.tensor.matmul(
                        ct_ps[b][:, h * 512:(h + 1) * 512],
                        lhsT=acols[:, j:j + 1],
                        rhs=eot[:, j, h * 512:(h + 1) * 512],
                        start=(j == 0), stop=(j == TJ - 1),
                        skip_group_check=True,
                    )
            ct_finish(b)

    nc.finalize()
    return nc


_CACHE: dict = {}


def _get_nc() -> bass.Bass:
    if "nc" not in _CACHE:
        _CACHE["nc"] = build_bass(NB)
    return _CACHE["nc"]


def make_in_maps(inputs: dict) -> list:
    f = lambda x: np.ascontiguousarray(np.asarray(x), dtype=np.float32)
    s = f(inputs["s_t_hat"])
    eo = f(inputs["encoder_outputs"])
    ef = f(inputs["encoder_feature"]).reshape(B, T, N)
    mk = f(inputs["enc_padding_mask"])
    cv = f(inputs["coverage"])
    fp8 = ml_dtypes.float8_e4m3fn
    bf = ml_dtypes.bfloat16
    wdt = np.ascontiguousarray(f(inputs["W_d"]).T).astype(fp8)
    bd = f(inputs["b_d"])
    wc = f(inputs["W_c"])
    vv = f(inputs["v"])
    # EF n-major: [b, p, j, t] with n = 128*j + p   (16KB partition lines)
    ef_b = np.ascontiguousarray(
        ef.astype(bf).reshape(B, T, TJ, 128).transpose(0, 3, 2, 1)
    )
    # EO t-major: [b, p, j, n] with t = 128*j + p
    eo_b = np.ascontiguousarray(
        eo.astype(bf).reshape(B, TJ, 128, N).transpose(0, 2, 1, 3)
    )
    vdg = np.zeros((128, TJ, 128), dtype=bf)
    for j in range(TJ):
        vdg[:, j, j] = vv[j * 128:(j + 1) * 128].astype(bf)
    in_maps = []
    for i in range(N_CORES):
        sl = slice(i * NB, (i + 1) * NB)
        in_maps.append({
            "eft_bf16": ef_b[sl],
            "eo_bf16": eo_b[sl],
            "s_t_hat_T": np.ascontiguousarray(s[sl].T).astype(fp8),
            "enc_padding_mask": mk[sl],
            "coverage": cv[sl],
            "W_d_T": wdt,
            "vdiag": vdg,
            "b_d": bd.astype(bf),
            "W_c": wc.astype(bf),
            "v": vv.astype(bf),
        })
    return in_maps


def gather_outputs(results: list):
    c_t = np.concatenate([results[i]["c_t"] for i in range(N_CORES)], axis=0)
    attn = np.concatenate([results[i]["attn"] for i in range(N_CORES)], axis=0)
    covn = np.concatenate(
        [results[i]["coverage_next"] for i in range(N_CORES)], axis=0
    )
    return c_t, attn, covn


def kernel(**inputs):
    nc = _get_nc()
    in_maps = make_in_maps(inputs)
    res = run_bass_kernel_spmd(nc, in_maps, core_ids=list(range(N_CORES)))
    return gather_outputs(res.results)


# revision 24
# speedup vs baseline: 1.1348x; 1.1348x over previous
"""Trainium2 Bass kernel for pointer-generator additive attention.

Full op (per batch b):
    dec_fea = s_t_hat @ W_d.T + b_d                         # (n,)
    att     = EF[b] + dec_fea[None,:] + cov[b][:,None]*W_c  # (t, n)
    score   = tanh(att) @ v                                 # (t,)
    attn    = renorm(softmax(score) * mask)                 # (t,)
    c_t     = attn @ EO[b]                                  # (n,)
    cov_next= cov + attn

Data-parallel over batch across 8 NeuronCores (8 batches/core, params
replicated, no collectives).

HBM strategy: EF (n-major transposed) and EO (t-major) are cast to bf16 and
permuted on the host into partition-contiguous layouts; each batch is ONE
2 MB HWDGE dma_start with 16 KB per-partition lines. The big stream owns the
nc.sync ring in FIFO need-order: W_d(fp8), EF0, EO0, EF1, ...
Total ~33 MB/core -> ~92 us DMA floor.

Engine split (att is n-major: partition = n, free = t):
  - DVE stt:  att_pre = EFT + W_c[n]*cov_bcast  (W_c as per-partition scalar)
  - ScalarE:  th = tanh(att_pre + dec[n])       (dec as per-partition bias)
  - PE score: M=1 matvecs (lhsT = v column per n-tile) accumulating a
    [1,1024] PSUM row over the 8 n-tiles.
  - PE c_t:   M=1 matvecs (lhsT = attn column, rhs = t-major EO tiles),
    interleaved tile-by-tile with the next batch's score matmuls.
  - softmax on an [8,128] grid: one exp (ScalarE), masked-sum via stt accum
    (DVE), total via a tiny PE ones-matmul -> [128,1] reciprocal, attn
    columns for c_t via one PE transpose of the unnormalized e-grid
    (short dependency chain so the c_t matmuls never head-block the PE).
dec_fea is a tiny fp8 PE matmul at init; cov broadcasts / dec columns are
prebuilt during the initial DMA ramp. GpSimd does only small DMAs (its
elementwise ops measured 10-25x slower than DVE and stall DVE via the
shared SBUF port; PE matvecs sit at the HAM-throttled 1.2 GHz clock since
M=1 registers almost no array activity - both measured dead ends).
"""

import sys

if "/opt/trn_rl_repo" not in sys.path:
    sys.path.insert(0, "/opt/trn_rl_repo")

import ml_dtypes
import numpy as np

import concourse.bass as bass
import concourse.bass_isa as bass_isa
import concourse.mybir as mybir
import concourse.tile as tile
from concourse import bacc
from concourse.bass_utils import run_bass_kernel_spmd
from concourse.masks import make_identity

F32 = mybir.dt.float32
BF16 = mybir.dt.bfloat16
FP8 = mybir.dt.float8e4
AF = mybir.ActivationFunctionType
ALU = mybir.AluOpType

N_CORES = 8
B = 64
NB = B // N_CORES  # local batches per core
T = 1024
N = 1024
TJ = T // 128       # 128-tiles per batch (both t- and n- direction)
KT = N // 128       # k-tiles for the W_d matvec


def build_bass(nb: int = NB) -> bass.Bass:
    nc = bacc.Bacc()

    eft_d = nc.declare_dram_parameter("eft_bf16", [nb, 128, TJ, T], BF16, isOutput=False)
    eo_d = nc.declare_dram_parameter("eo_bf16", [nb, 128, TJ, N], BF16, isOutput=False)
    mk_d = nc.declare_dram_parameter("enc_padding_mask", [nb, T], F32, isOutput=False)
    cv_d = nc.declare_dram_parameter("coverage", [nb, T], F32, isOutput=False)
    wdt_d = nc.declare_dram_parameter("W_d_T", [N, N], FP8, isOutput=False)
    st_d = nc.declare_dram_parameter("s_t_hat_T", [N, nb], FP8, isOutput=False)
    bd_d = nc.declare_dram_parameter("b_d", [N], BF16, isOutput=False)
    wc_d = nc.declare_dram_parameter("W_c", [N], BF16, isOutput=False)
    v_d = nc.declare_dram_parameter("v", [N], BF16, isOutput=False)
    vdg_d = nc.declare_dram_parameter("vdiag", [128, TJ, 128], BF16, isOutput=False)
    ct_o = nc.declare_dram_parameter("c_t", [nb, N], F32, isOutput=True)
    at_o = nc.declare_dram_parameter("attn", [nb, T], F32, isOutput=True)
    cn_o = nc.declare_dram_parameter("coverage_next", [nb, T], F32, isOutput=True)

    with tile.TileContext(nc) as tc:
        with (
            tc.tile_pool(name="consts", bufs=1) as consts,
            tc.tile_pool(name="wdtp", bufs=1) as wdtp,
            tc.tile_pool(name="covp", bufs=1) as covp,
            tc.tile_pool(name="efp", bufs=3) as efp,
            tc.tile_pool(name="eop", bufs=3) as eop,
            tc.tile_pool(name="attp", bufs=3) as attp,
            tc.tile_pool(name="thp", bufs=3) as thp,
            tc.tile_pool(name="smal", bufs=4) as smal,
            tc.tile_pool(name="rowstg", bufs=2) as rowstg,
            tc.tile_pool(name="psP", bufs=2, space="PSUM") as psP,
            tc.tile_pool(name="psC", bufs=1, space="PSUM") as psC,
            tc.tile_pool(name="psT", bufs=1, space="PSUM") as psT,
        ):
            # ---------------- big-stream DMAs (sync ring, FIFO need-order) ----
            wdt_all = wdtp.tile([128, KT, N], FP8)
            nc.sync.dma_start(
                out=wdt_all, in_=wdt_d.rearrange("(kj p) n -> p kj n", p=128)
            )
            ef_bufs = {}
            eo_bufs = {}

            def load_ef(b, split=1):
                t = efp.tile([128, TJ, T], BF16, tag="ef", name=f"ef{b}")
                step = TJ // split
                for s in range(split):
                    nc.sync.dma_start(
                        out=t[:, s * step:(s + 1) * step, :],
                        in_=eft_d[b, :, s * step:(s + 1) * step, :],
                    )
                ef_bufs[b] = t

            def load_eo(b):
                t = eop.tile([128, TJ, N], BF16, tag="eo", name=f"eo{b}")
                nc.sync.dma_start(out=t, in_=eo_d[b])
                eo_bufs[b] = t

            load_ef(0, split=2)
            load_eo(0)
            load_ef(1)
            load_eo(1)

            # ---------------- constants / small inputs (gpsimd ring) ----------
            ident = consts.tile([128, 128], F32)
            make_identity(nc, ident)
            ident_b = consts.tile([128, 128], BF16)
            nc.vector.tensor_copy(ident_b, ident)
            ones_f32 = consts.tile([1, T], F32)
            nc.vector.memset(ones_f32, 1.0)
            ones_b = consts.tile([1, T], BF16)
            nc.vector.memset(ones_b, 1.0)
            ones8w = consts.tile([TJ, 128], F32)
            nc.vector.memset(ones8w, 1.0)
            ones8b = consts.tile([TJ, 1], BF16)
            nc.vector.memset(ones8b, 1.0)

            sT_all = consts.tile([128, KT, 32], FP8)
            nc.gpsimd.dma_start(
                out=sT_all[:, :, 0:nb],
                in_=st_d.rearrange("(kj p) b -> p kj b", p=128),
            )
            bd_b = consts.tile([1, N], BF16)
            nc.gpsimd.dma_start(out=bd_b, in_=bd_d[None, :])
            v8 = consts.tile([TJ, 128], BF16)
            nc.gpsimd.dma_start(out=v8, in_=v_d.rearrange("(j t) -> j t", j=TJ))
            wc8 = consts.tile([TJ, 128], BF16)
            nc.gpsimd.dma_start(out=wc8, in_=wc_d.rearrange("(j t) -> j t", j=TJ))
            mk8_all = consts.tile([TJ, nb, 128], F32)
            nc.gpsimd.dma_start(
                out=mk8_all, in_=mk_d.rearrange("b (j t) -> j b t", j=TJ)
            )
            cov8_all = consts.tile([TJ, nb, 128], F32)
            nc.gpsimd.dma_start(
                out=cov8_all, in_=cv_d.rearrange("b (j t) -> j b t", j=TJ)
            )

            # v / W_c as per-partition columns: transpose [8,128] -> [128,8]
            v_cols = consts.tile([128, TJ], BF16)
            wc_cols = consts.tile([128, TJ], BF16)
            for src, dst in ((v8, v_cols), (wc8, wc_cols)):
                pst = psT.tile([128, TJ], BF16, tag="tscratchb")
                nc.tensor.matmul(
                    pst, lhsT=src, rhs=ident_b[0:TJ, 0:TJ], is_transpose=True,
                    start=True, stop=True,
                )
                nc.vector.tensor_copy(dst, pst)

            # vdiag[j] = [128,128] with only column j nonzero (the v-slice):
            # an M=128 stationary operand so score matvecs register full
            # array activity (keeps the HAM clock at 8/8)


            # dec_fea = s_t_hat @ W_d.T + b_d  (fp8 matmuls, tiny), then
            # transpose into per-partition columns dec_cols[:, j, b]
            dec_cols = consts.tile([128, TJ, nb], F32)
            for h in range(2):
                sl = slice(h * 512, (h + 1) * 512)
                psd = psT.tile([nb, 512], F32, tag="tscratch")
                for kj in range(KT):
                    nc.tensor.matmul(
                        psd, lhsT=sT_all[:, kj, 0:nb], rhs=wdt_all[:, kj, sl],
                        start=(kj == 0), stop=False,
                    )
                nc.tensor.matmul(
                    psd, lhsT=ones_b[0:1, 0:nb], rhs=bd_b[0:1, sl],
                    start=False, stop=True,
                )
                dec8 = smal.tile([nb, 512], F32, tag="dec8")
                nc.vector.tensor_copy(dec8, psd)
                for jj in range(4):
                    j = h * 4 + jj
                    pst = psT.tile([128, TJ], F32, tag="tscratch")
                    nc.tensor.matmul(
                        pst[:, 0:nb], lhsT=dec8[:, jj * 128:(jj + 1) * 128],
                        rhs=ident[0:nb, 0:nb], is_transpose=True,
                        start=True, stop=True,
                    )
                    nc.vector.tensor_copy(dec_cols[:, j, :], pst[:, 0:nb])

            # cov broadcast tiles for every batch (built during the DMA ramp)
            covb_all = covp.tile([128, nb, T], BF16)
            for b in range(nb):
                covrow = smal.tile([1, T], BF16, tag="covrow", name=f"cvr{b}")
                nc.gpsimd.dma_start(out=covrow, in_=cv_d[None, b, :])  # cast
                for h in range(2):
                    cps = psT.tile([128, 512], F32, tag="tscratch")
                    nc.tensor.matmul(
                        cps, lhsT=ones_b[0:1, 0:128],
                        rhs=covrow[0:1, h * 512:(h + 1) * 512],
                        start=True, stop=True,
                    )
                    nc.vector.tensor_copy(
                        covb_all[:, b, h * 512:(h + 1) * 512], cps
                    )

            # ---------------- main loop over local batches ----------------
            attn_cols = {}
            ct_ps = {}

            def ct_finish(b):
                ps = ct_ps.pop(b)
                ct_row = rowstg.tile([1, N], BF16, tag="ctb", name=f"ct{b}")
                nc.vector.tensor_copy(ct_row[:, 0:512], ps[:, 0:512])
                nc.vector.tensor_copy(ct_row[:, 512:1024], ps[:, 512:1024])
                nc.gpsimd.dma_start(out=ct_o[None, b, :], in_=ct_row)
                eo_bufs.pop(b)

            def softmax_block(b, spart):
                score_row = rowstg.tile([1, T], BF16, tag="score", name=f"sc{b}")
                nc.vector.tensor_copy(score_row[:, 0:512], spart[:, 0:512])
                nc.vector.tensor_copy(score_row[:, 512:1024], spart[:, 512:1024])
                score8 = smal.tile([TJ, 128], BF16, tag="s8")
                nc.gpsimd.dma_start(
                    out=score8,
                    in_=score_row[0:1, :].rearrange("p (j t) -> p j t", j=TJ),
                )
                e8 = smal.tile([TJ, 128], F32, tag="e8")
                # scores are O(1) (|s| < ~3): plain exp is safe, skip max-sub
                nc.scalar.activation(e8, score8, AF.Exp)
                e8m = smal.tile([TJ, 128], F32, tag="e8m")
                msum8 = smal.tile([TJ, 1], F32, tag="msum")
                nc.vector.scalar_tensor_tensor(
                    out=e8m, in0=e8, scalar=1.0, in1=mk8_all[:, b, :],
                    op0=ALU.mult, op1=ALU.mult, accum_out=msum8,
                )
                # S via gpsimd cross-partition sum (keeps the PE free of
                # f32 matmuls + their weight loads), then 1/S broadcast
                rsum8 = smal.tile([TJ, 1], F32, tag="rsum")
                nc.gpsimd.partition_all_reduce(
                    rsum8, msum8, channels=TJ,
                    reduce_op=bass_isa.ReduceOp.add,
                )
                rs8 = smal.tile([TJ, 1], F32, tag="rs8")
                nc.vector.reciprocal(rs8, rsum8)
                rs128 = smal.tile([128, 1], F32, tag="rs128")
                nc.gpsimd.partition_broadcast(rs128, rs8[0:1, :])
                # attn columns for c_t: transpose unnormalized e-grid, scale
                ecp = psT.tile([128, TJ], F32, tag="tscratch")
                nc.tensor.matmul(
                    ecp, lhsT=e8m, rhs=ident[0:TJ, 0:TJ], is_transpose=True,
                    start=True, stop=True,
                )
                acols = smal.tile([128, TJ], BF16, tag="acols")
                nc.vector.tensor_scalar_mul(acols, ecp, rs128)
                attn_cols[b] = acols
                # attn / coverage outputs
                attn8 = smal.tile([TJ, 128], F32, tag="attn8")
                nc.vector.tensor_scalar_mul(attn8, e8m, rs8)
                covn8 = smal.tile([TJ, 128], F32, tag="covn8")
                nc.vector.tensor_add(covn8, cov8_all[:, b, :], attn8)
                nc.gpsimd.dma_start(
                    out=at_o[b].rearrange("(j t) -> j t", j=TJ), in_=attn8
                )
                nc.gpsimd.dma_start(
                    out=cn_o[b].rearrange("(j t) -> j t", j=TJ), in_=covn8
                )

            for b in range(nb):
                eft = ef_bufs.pop(b)
                if b + 2 < nb:
                    load_ef(b + 2)
                    load_eo(b + 2)
                spart = psP.tile([1, T], F32, tag="spart", name=f"spart{b}")
                if b > 0:
                    ct_ps[b - 1] = psC.tile([1, N], F32, tag="ctp",
                                            name=f"ctp{b}")
                for j in range(TJ):
                    att_pre = attp.tile([128, T], BF16, tag="attp")
                    nc.vector.scalar_tensor_tensor(
                        out=att_pre, in0=covb_all[:, b, :],
                        scalar=wc_cols[:, j:j + 1], in1=eft[:, j, :],
                        op0=ALU.mult, op1=ALU.add,
                    )
                    th = thp.tile([128, T], BF16, tag="th")
                    nc.scalar.activation(
                        th, att_pre, AF.Tanh, bias=dec_cols[:, j, b:b + 1]
                    )
                    for h in range(2):
                        nc.tensor.matmul(
                            spart[:, h * 512:(h + 1) * 512],
                            lhsT=v_cols[:, j:j + 1],
                            rhs=th[:, h * 512:(h + 1) * 512],
                            start=(j == 0), stop=(j == TJ - 1),
                            skip_group_check=True,
                        )
                    if b > 0:
                        acols = attn_cols[b - 1]
                        eot = eo_bufs[b - 1]
                        for h in range(2):
                            nc.tensor.matmul(
                                ct_ps[b - 1][:, h * 512:(h + 1) * 512],
                                lhsT=acols[:, j:j + 1],
                                rhs=eot[:, j, h * 512:(h + 1) * 512],
                                start=(j == 0), stop=(j == TJ - 1),
                                skip_group_check=True,
                            )
                softmax_block(b, spart)
                if b > 0:
                    ct_finish(b - 1)

            # trailing c_t for the last batch
            b = nb - 1
            ct_ps[b] = psC.tile([1, N], F32, tag="ctp", name="ctp_last")
            acols = attn_cols[b]
            eot = eo_bufs[b]
            for j in range(TJ):
                for h in range(2):
                    nc.tensor.matmul(
                        ct_ps[b][:, h * 512:(h + 1) * 512],
                        lhsT=acols[:, j:j + 1],
                        rhs=eot[:, j, h * 512:(h + 1) * 512],
                        start=(j == 0), stop=(j == TJ - 1),
                        skip_group_check=True,
                    )
            ct_finish(b)

    nc.finalize()
    return nc


_CACHE: dict = {}


def _get_nc() -> bass.Bass:
    if "nc" not in _CACHE:
        _CACHE["nc"] = build_bass(NB)
    return _CACHE["nc"]


def make_in_maps(inputs: dict) -> list:
    f = lambda x: np.ascontiguousarray(np.asarray(x), dtype=np.float32)
    s = f(inputs["s_t_hat"])
    eo = f(inputs["encoder_outputs"])
    ef = f(inputs["encoder_feature"]).reshape(B, T, N)
    mk = f(inputs["enc_padding_mask"])
    cv = f(inputs["coverage"])
    fp8 = ml_dtypes.float8_e4m3fn
    bf = ml_dtypes.bfloat16
    wdt = np.ascontiguousarray(f(inputs["W_d"]).T).astype(fp8)
    bd = f(inputs["b_d"])
    wc = f(inputs["W_c"])
    vv = f(inputs["v"])
    # EF n-major: [b, p, j, t] with n = 128*j + p   (16KB partition lines)
    ef_b = np.ascontiguousarray(
        ef.astype(bf).reshape(B, T, TJ, 128).transpose(0, 3, 2, 1)
    )
    # EO t-major: [b, p, j, n] with t = 128*j + p
    eo_b = np.ascontiguousarray(
        eo.astype(bf).reshape(B, TJ, 128, N).transpose(0, 2, 1, 3)
    )
    vdg = np.zeros((128, TJ, 128), dtype=bf)
    for j in range(TJ):
        vdg[:, j, j] = vv[j * 128:(j + 1) * 128].astype(bf)
    in_maps = []
    for i in range(N_CORES):
        sl = slice(i * NB, (i + 1) * NB)
        in_maps.append({
            "eft_bf16": ef_b[sl],
            "eo_bf16": eo_b[sl],
            "s_t_hat_T": np.ascontiguousarray(s[sl].T).astype(fp8),
            "enc_padding_mask": mk[sl],
            "coverage": cv[sl],
            "W_d_T": wdt,
            "vdiag": vdg,
            "b_d": bd.astype(bf),
            "W_c": wc.astype(bf),
            "v": vv.astype(bf),
        })
    return in_maps


def gather_outputs(results: list):
    c_t = np.concatenate([results[i]["c_t"] for i in range(N_CORES)], axis=0)
    attn = np.concatenate([results[i]["attn"] for i in range(N_CORES)], axis=0)
    covn = np.concatenate(
        [results[i]["coverage_next"] for i in range(N_CORES)], axis=0
    )
    return c_t, attn, covn


def kernel(**inputs):
    nc = _get_nc()
    in_maps = make_in_maps(inputs)
    res = run_bass_kernel_spmd(nc, in_maps, core_ids=list(range(N_CORES)))
    return gather_outputs(res.results)
